# revision 10
# baseline (speedup 1.0000x reference)
"""Trainium2 Bass kernel for a diagonal selective SSM layer.

Reference computation (per batch element b):
    alpha = sigmoid(x @ Wg.T + bg)        # (L, S)
    u     = x @ WB.T + bB                 # (L, S)
    h_t   = alpha_t * h_{t-1} + u_t       # scan over L, h in R^S
    y     = h @ WC.T + bC                 # (L, D)

Sharding: data-parallel over batch. B == 8 == n_cores, so each NeuronCore
processes exactly one batch element; the small projection weights are
replicated to every core. No collectives needed.

Per-core dataflow:
  - Gate GEMM (alpha) in fp8 e4m3 with MatmulPerfMode.DoubleRow: the PE
    packs two fp8 k-rows per cell, contracting K=256 per instruction
    (~1.44x bf16 throughput at N>=256).  Wg ships pre-scaled by 32 so its
    ~N(0, 1/32) entries use the fp8 dynamic range; the sigmoid eviction
    folds the inverse scale (out = sigmoid(psum/32 + bg)).  x ships both
    as fp8 (gate GEMM) and bf16 (input GEMM) - the extra 1B/elem of DMA
    is cheaper than an on-chip cast.
  - U GEMM and output GEMM stay bf16 (fp8 there pushes rel-err past the
    2e-2 budget; gate-only fp8 lands ~1e-2 because sigmoid's derivative
    shrinks the quantization noise ~4x).
  - All DRAM operands are shipped in SBUF layout with chunk-major
    blocking so every DMA issue is a plain 2D contiguous copy with 2-8KB
    per-partition lines.
  - Input DMAs go on one HWDGE queue in exact consumption order; the
    first transfers are split in halves so the PE can start consuming a
    chunk while its second half is still in flight.
  - Recurrence: hardware linear-recurrence nc.vector.tensor_tensor_scan
    (state = a*state + u, fp32 internal state), chunk-chained via
    `initial`.
  - Output GEMM transposed (yT layout, D on partitions): the scan output
    hh (S on partitions, L free) is the moving operand, WC tiles (S
    parts, D free) the stationary one.  bias bC fuses into the PSUM
    eviction (split ScalarE/VectorE) which also casts to bf16.
  - HAM warm-up: a short burst of dummy matmuls bridges the engine
    preamble to first-data so the PE activity window stays hot; the
    early chunks then run DMA-paced, which keeps ramping the clock gate.
  - Y GEMMs skew one chunk behind the G/U GEMMs so the PE never waits on
    the scan; the last two chunks are small to shorten the tail.
"""

import numpy as np

B, L, D, S = 8, 2048, 1024, 256
P = 128
NCORES = 8
KD = D // P      # 8 k-tiles over the D contraction
KP = KD // 2     # 4 fp8 DoubleRow k-pairs
MS = S // P      # 2 partition groups over S
DT = D // P      # 8 output D-tiles

# Small first chunk (first-data lands sooner through the slow early DMA
# ramp), big middle, small last chunk (short scan->Y->writeback tail).
CHUNKS = [256, 512, 512, 512, 256]
OFFS = [sum(CHUNKS[:i]) for i in range(len(CHUNKS) + 1)]
XOFF = [KD * o for o in OFFS]   # x/x8 block offsets
YOFF = [DT * o for o in OFFS]   # y block offsets ([q][t][l] layout)
assert OFFS[-1] == L

WARMUP_MMS = 24  # N=128 dummy matmuls bridging preamble-end to first-data
WG_SCALE = 32.0  # Wg pre-scale before fp8 quantization (undone in eviction)

_NC_CACHE = {}


def _build_nc():
    import concourse.mybir as mybir
    import concourse.tile as tile
    from concourse import bacc

    f32 = mybir.dt.float32
    bf16 = mybir.dt.bfloat16
    f8 = mybir.dt.float8e4
    AF = mybir.ActivationFunctionType
    OP = mybir.AluOpType
    DR = mybir.MatmulPerfMode.DoubleRow

    nc = bacc.Bacc("TRN2", target_bir_lowering=False, debug=False)

    x8Q = nc.dram_tensor("x8Q", [P, KD * L], f8, kind="ExternalInput")
    xQ = nc.dram_tensor("xQ", [P, KD * L], bf16, kind="ExternalInput")
    wg = nc.dram_tensor("wg8P", [P, KD * S], f8, kind="ExternalInput")
    wb = nc.dram_tensor("wbP", [P, KD * S], bf16, kind="ExternalInput")
    wc = nc.dram_tensor("wcP", [P, MS * D], bf16, kind="ExternalInput")
    bias = nc.dram_tensor("biasP", [P, 4 + DT], f32, kind="ExternalInput")
    y = nc.dram_tensor("yQ", [P, DT * L], bf16, kind="ExternalOutput")

    MH = KD * P  # one m-half of a G/U weight tile (1024 cols)

    with tile.TileContext(nc) as tc:
        with (
            tc.tile_pool(name="persist", bufs=1) as pp,
            tc.tile_pool(name="psum", bufs=8, space="PSUM") as psp,
        ):
            wgta = pp.tile([P, KD * S], f8, name="wgta", tag="wgta")
            wbta = pp.tile([P, KD * S], bf16, name="wbta", tag="wbta")
            wcta = pp.tile([P, MS * D], bf16, name="wcta", tag="wcta")
            biast = pp.tile([P, 4 + DT], f32, name="biast", tag="biast")
            xs8 = pp.tile([P, KD * L], f8, name="xs8", tag="xs8")
            xsa = pp.tile([P, KD * L], bf16, name="xsa", tag="xsa")
            ysta = pp.tile([P, DT * L], bf16, name="ysta", tag="ysta")

            # PE warm-up fodder (no DMA dependencies)
            wul = pp.tile([P, P], bf16, name="wul", tag="wul")

            # Input DMAs: one queue, exact consumption order, first
            # transfers halved so compute can chase the DMA.
            def dma_x8(q, parts=1):
                a, b = XOFF[q], XOFF[q + 1]
                step = (b - a) // parts
                for i in range(parts):
                    nc.sync.dma_start(
                        xs8[:, a + i * step:a + (i + 1) * step],
                        x8Q[:, a + i * step:a + (i + 1) * step])

            def dma_xb(q, parts=1):
                a, b = XOFF[q], XOFF[q + 1]
                step = (b - a) // parts
                for i in range(parts):
                    nc.sync.dma_start(
                        xsa[:, a + i * step:a + (i + 1) * step],
                        xQ[:, a + i * step:a + (i + 1) * step])

            # One HWDGE ring in exact consumption order: the 16 DMA
            # engines are a shared ~350-420 GB/s pipe (with a slow ~5us
            # ramp at kernel start), so parallel rings only dilute the
            # critical head transfers.  Only the tiny bias rides the
            # Scalar ring.
            nc.scalar.dma_start(biast[:], bias[:, :])
            nc.sync.dma_start(wgta[:, :MH], wg[:, :MH])
            dma_x8(0)
            nc.sync.dma_start(wgta[:, MH:], wg[:, MH:])
            nc.sync.dma_start(wbta[:, :MH], wb[:, :MH])
            dma_xb(0)
            nc.sync.dma_start(wbta[:, MH:], wb[:, MH:])
            dma_x8(1)
            nc.sync.dma_start(wcta[:], wc[:, :])
            dma_xb(1, parts=2)
            for q in range(2, len(CHUNKS)):
                dma_x8(q)
                dma_xb(q)

            alpha = [pp.tile([P, L], f32, name=f"al{m}", tag=f"al{m}") for m in range(MS)]
            uu = [pp.tile([P, L], f32, name=f"uu{m}", tag=f"uu{m}") for m in range(MS)]
            hh = [pp.tile([P, L], bf16, name=f"hh{m}", tag=f"hh{m}") for m in range(MS)]

            if WARMUP_MMS:
                nc.gpsimd.memset(wul[:], 0.0)
                wps = psp.tile([P, 512], f32, name="wps", tag="ps")
                for i in range(WARMUP_MMS):
                    nc.tensor.matmul(
                        wps[:, :P], wul[:], wul[:],
                        start=(i == 0), stop=(i == WARMUP_MMS - 1),
                    )

            def emit_g(q):
                # gate GEMM: fp8 DoubleRow, K=256 per matmul
                o0, o1 = OFFS[q], OFFS[q + 1]
                cl = o1 - o0
                qs = slice(o0, o1)
                for m in range(MS):
                    ps = psp.tile([P, 512], f32, name="ps", tag="ps")
                    for kp in range(KP):
                        lhsT = wgta[:, m * MH + kp * 2 * P:m * MH + (kp + 1) * 2 * P
                                    ].rearrange("p (two m) -> p two m", two=2)
                        rhs = xs8[:, XOFF[q] + kp * 2 * cl:XOFF[q] + (kp + 1) * 2 * cl
                                  ].rearrange("p (two n) -> p two n", two=2)
                        nc.tensor.matmul(
                            ps[:, :cl], lhsT, rhs,
                            start=(kp == 0), stop=(kp == KP - 1),
                            perf_mode=DR,
                        )
                    nc.scalar.activation(
                        alpha[m][:, qs], ps[:, :cl], AF.Sigmoid,
                        bias=biast[:, m:m + 1], scale=1.0 / WG_SCALE,
                    )

            def emit_u(q):
                # input GEMM (bf16) + chunk-chained hardware scan
                o0, o1 = OFFS[q], OFFS[q + 1]
                cl = o1 - o0
                qs = slice(o0, o1)
                for m in range(MS):
                    ps = psp.tile([P, 512], f32, name="ps", tag="ps")
                    for k in range(KD):
                        nc.tensor.matmul(
                            ps[:, :cl],
                            wbta[:, m * MH + k * P:m * MH + (k + 1) * P],
                            xsa[:, XOFF[q] + k * cl:XOFF[q] + (k + 1) * cl],
                            start=(k == 0),
                            stop=(k == KD - 1),
                        )
                    nc.vector.tensor_scalar_add(
                        uu[m][:, qs], ps[:, :cl], biast[:, 2 + m:3 + m],
                    )
                # state = alpha*state + u
                for m in range(MS):
                    init = 0.0 if q == 0 else hh[m][:, o0 - 1:o0]
                    nc.vector.tensor_tensor_scan(
                        hh[m][:, qs], alpha[m][:, qs], uu[m][:, qs],
                        init, OP.mult, OP.add,
                    )

            def emit_y(q):
                o0, o1 = OFFS[q], OFFS[q + 1]
                cl = o1 - o0
                qs = slice(o0, o1)
                last = q == len(CHUNKS) - 1
                for t in range(DT):
                    ps = psp.tile([P, 512], f32, name="psy", tag="ps")
                    for m in range(MS):
                        nc.tensor.matmul(
                            ps[:, :cl],
                            wcta[:, m * D + t * P:m * D + (t + 1) * P],
                            hh[m][:, qs],
                            start=(m == 0),
                            stop=(m == MS - 1),
                        )
                    dst = ysta[:, YOFF[q] + t * cl:YOFF[q] + (t + 1) * cl]
                    bc = biast[:, 4 + t:5 + t]
                    if last and t == DT - 1:
                        # the very last eviction gates the final writeback:
                        # split it across both engines so it lands sooner
                        hl = cl // 2
                        nc.scalar.activation(
                            dst[:, :hl], ps[:, :hl], AF.Identity, bias=bc, scale=1.0
                        )
                        nc.vector.tensor_scalar_add(dst[:, hl:], ps[:, hl:cl], bc)
                    elif t % 2 == 0:
                        nc.scalar.activation(dst, ps[:, :cl], AF.Identity, bias=bc, scale=1.0)
                    else:
                        nc.vector.tensor_scalar_add(dst, ps[:, :cl], bc)
                    if t == DT // 2 - 1:
                        # first-half writeback starts while the second half
                        # of this chunk's Y GEMMs still run
                        nc.sync.dma_start(
                            y[:, YOFF[q]:YOFF[q] + DT // 2 * cl],
                            ysta[:, YOFF[q]:YOFF[q] + DT // 2 * cl],
                        )
                # final chunk: second half goes out on the otherwise-idle
                # Scalar HWDGE ring so the two halves drain in parallel
                eng = nc.scalar if q == len(CHUNKS) - 1 else nc.sync
                eng.dma_start(
                    y[:, YOFF[q] + DT // 2 * cl:YOFF[q + 1]],
                    ysta[:, YOFF[q] + DT // 2 * cl:YOFF[q + 1]],
                )

            # software pipeline: Y GEMMs run one chunk behind the G/U
            # GEMMs (never waiting on the scan), interleaved between U and
            # G so Y work can fill x-DMA wait gaps.  The final chunk's Y
            # is pulled as close to its scan as possible.
            # PE order: g0 u0 g1 y0 u1 g2 y1 u2 g3 y2 u3 y3 g4 u4 y4
            nq = len(CHUNKS)
            emit_g(0)
            emit_u(0)
            emit_g(1)
            for q in range(1, nq - 1):
                emit_y(q - 1)
                emit_u(q)
                if q + 1 <= nq - 2:
                    emit_g(q + 1)
            emit_y(nq - 2)
            emit_g(nq - 1)
            emit_u(nq - 1)
            emit_y(nq - 1)

    nc.finalize()
    return nc


def _get_nc():
    if "nc" not in _NC_CACHE:
        _NC_CACHE["nc"] = _build_nc()
    return _NC_CACHE["nc"]


def _make_in_maps(x, Wg, bg, WB, bB, WC, bC):
    import ml_dtypes

    bf16 = ml_dtypes.bfloat16
    f8 = ml_dtypes.float8_e4m3
    x = np.asarray(x, dtype=np.float32)
    # fp8 gate weights, DoubleRow layout: (p, m, kp, two, j) ->
    # Wg.T[(2kp+two)*P+p, m*P+j] * WG_SCALE
    wg8P = np.ascontiguousarray(
        np.clip(np.asarray(Wg, dtype=np.float32).T * WG_SCALE, -240, 240)
        .astype(f8)
        .reshape(KP, 2, P, MS, P).transpose(2, 3, 0, 1, 4).reshape(P, KD * S)
    )
    # bf16 U weights m-major: (p, m, k, j) -> WB.T[k*P+p, m*P+j]
    wbP = np.ascontiguousarray(
        np.asarray(WB, dtype=np.float32).T.astype(bf16)
        .reshape(KD, P, MS, P).transpose(1, 2, 0, 3).reshape(P, MS * KD * P)
    )
    wcP = np.ascontiguousarray(
        np.asarray(WC, dtype=np.float32).T.astype(bf16)
        .reshape(MS, P, D).transpose(1, 0, 2).reshape(P, MS * D)
    )
    bias = np.zeros((P, 4 + DT), dtype=np.float32)
    bias[:, 0] = np.asarray(bg, dtype=np.float32)[0:P]
    bias[:, 1] = np.asarray(bg, dtype=np.float32)[P:2 * P]
    bias[:, 2] = np.asarray(bB, dtype=np.float32)[0:P]
    bias[:, 3] = np.asarray(bB, dtype=np.float32)[P:2 * P]
    bias[:, 4:] = np.asarray(bC, dtype=np.float32).reshape(DT, P).T
    in_maps = []
    for b in range(NCORES):
        xt = np.ascontiguousarray(x[b].T)          # [D, L] f32
        xkb = xt.astype(bf16).reshape(KD, P, L)    # [k, p, l]
        xk8 = np.clip(xt, -240, 240).astype(f8).reshape(KD, P, L)
        bb, b8 = [], []
        for q in range(len(CHUNKS)):
            sl = slice(OFFS[q], OFFS[q + 1])
            # bf16 block: (p, k, l)
            bb.append(xkb[:, :, sl].transpose(1, 0, 2).reshape(P, -1))
            # fp8 block: (p, kp, two, l)
            b8.append(
                xk8[:, :, sl].reshape(KP, 2, P, -1).transpose(2, 0, 1, 3).reshape(P, -1)
            )
        in_maps.append({
            "xQ": np.ascontiguousarray(np.concatenate(bb, axis=1)),
            "x8Q": np.ascontiguousarray(np.concatenate(b8, axis=1)),
            "wg8P": wg8P,
            "wbP": wbP,
            "wcP": wcP,
            "biasP": bias,
        })
    return in_maps


def _run(in_maps, **kwargs):
    from concourse.bass_utils import run_bass_kernel_spmd

    nc = _get_nc()
    return run_bass_kernel_spmd(nc, in_maps, list(range(NCORES)), **kwargs)


def kernel(x, Wg, bg, WB, bB, WC, bC):
    res = _run(_make_in_maps(x, Wg, bg, WB, bB, WC, bC))
    out = np.empty((NCORES, L, D), dtype=np.float32)
    for b in range(NCORES):
        yq = np.asarray(res.results[b]["yQ"])
        for q in range(len(CHUNKS)):
            o0, o1 = OFFS[q], OFFS[q + 1]
            cl = o1 - o0
            blk = yq[:, YOFF[q]:YOFF[q + 1]].reshape(P, DT, cl)
            # yQ[p, t, l] = y[o0+l, t*P+p]
            out[b, o0:o1, :] = blk.transpose(2, 1, 0).reshape(cl, D).astype(np.float32)
    return out


# revision 11
# speedup vs baseline: 1.0588x; 1.0588x over previous
"""Trainium2 Bass kernel for a diagonal selective SSM layer.

Reference computation (per batch element b):
    alpha = sigmoid(x @ Wg.T + bg)        # (L, S)
    u     = x @ WB.T + bB                 # (L, S)
    h_t   = alpha_t * h_{t-1} + u_t       # scan over L, h in R^S
    y     = h @ WC.T + bC                 # (L, D)

Sharding: data-parallel over batch. B == 8 == n_cores, so each NeuronCore
processes exactly one batch element; the small projection weights are
replicated to every core. No collectives needed.

Per-core dataflow:
  - Gate GEMM (alpha) in fp8 e4m3 with MatmulPerfMode.DoubleRow: the PE
    packs two fp8 k-rows per cell, contracting K=256 per instruction
    (~1.44x bf16 throughput at N>=256).  Wg ships pre-scaled by 32 so its
    ~N(0, 1/32) entries use the fp8 dynamic range; the sigmoid eviction
    folds the inverse scale (out = sigmoid(psum/32 + bg)).  x ships both
    as fp8 (gate GEMM) and bf16 (input GEMM) - the extra 1B/elem of DMA
    is cheaper than an on-chip cast.
  - U GEMM and output GEMM stay bf16 (fp8 there pushes rel-err past the
    2e-2 budget; gate-only fp8 lands ~1e-2 because sigmoid's derivative
    shrinks the quantization noise ~4x).
  - All DRAM operands are shipped in SBUF layout with chunk-major
    blocking so every DMA issue is a plain 2D contiguous copy with 2-8KB
    per-partition lines.
  - Input DMAs go on one HWDGE queue in exact consumption order; the
    first transfers are split in halves so the PE can start consuming a
    chunk while its second half is still in flight.
  - Recurrence: hardware linear-recurrence nc.vector.tensor_tensor_scan
    (state = a*state + u, fp32 internal state), chunk-chained via
    `initial`.
  - Output GEMM transposed (yT layout, D on partitions): the scan output
    hh (S on partitions, L free) is the moving operand, WC tiles (S
    parts, D free) the stationary one.  bias bC fuses into the PSUM
    eviction (split ScalarE/VectorE) which also casts to bf16.
  - HAM warm-up: a short burst of dummy matmuls bridges the engine
    preamble to first-data so the PE activity window stays hot; the
    early chunks then run DMA-paced, which keeps ramping the clock gate.
  - Y GEMMs skew one chunk behind the G/U GEMMs so the PE never waits on
    the scan; the last two chunks are small to shorten the tail.
"""

import numpy as np

B, L, D, S = 8, 2048, 1024, 256
P = 128
NCORES = 8
KD = D // P      # 8 k-tiles over the D contraction
KP = KD // 2     # 4 fp8 DoubleRow k-pairs
MS = S // P      # 2 partition groups over S
DT = D // P      # 8 output D-tiles

# First chunk large: the early phase is DMA-ramp-bound regardless, and a
# long first chunk gives the PE a work backlog so it never idles into a
# HAM re-throttle.  Last chunks small to shorten the tail.
CHUNKS = [512, 512, 512, 256, 256]
OFFS = [sum(CHUNKS[:i]) for i in range(len(CHUNKS) + 1)]
XOFF = [KD * o for o in OFFS]   # x/x8 block offsets
YOFF = [DT * o for o in OFFS]   # y block offsets ([q][t][l] layout)
assert OFFS[-1] == L

WARMUP_MMS = 16  # N=128 dummy matmuls bridging preamble-end to first-data
WG_SCALE = 32.0  # Wg pre-scale before fp8 quantization (undone in eviction)

_NC_CACHE = {}


def _build_nc():
    import concourse.mybir as mybir
    import concourse.tile as tile
    from concourse import bacc

    f32 = mybir.dt.float32
    bf16 = mybir.dt.bfloat16
    f8 = mybir.dt.float8e4
    AF = mybir.ActivationFunctionType
    OP = mybir.AluOpType
    DR = mybir.MatmulPerfMode.DoubleRow

    nc = bacc.Bacc("TRN2", target_bir_lowering=False, debug=False)

    x8Q = nc.dram_tensor("x8Q", [P, KD * L], f8, kind="ExternalInput")
    xQ = nc.dram_tensor("xQ", [P, KD * L], bf16, kind="ExternalInput")
    wg = nc.dram_tensor("wg8P", [P, KD * S], f8, kind="ExternalInput")
    wb = nc.dram_tensor("wbP", [P, KD * S], bf16, kind="ExternalInput")
    wc = nc.dram_tensor("wcP", [P, MS * D], bf16, kind="ExternalInput")
    bias = nc.dram_tensor("biasP", [P, 4 + DT], f32, kind="ExternalInput")
    y = nc.dram_tensor("yQ", [P, DT * L], bf16, kind="ExternalOutput")

    MH = KD * P  # one m-half of a G/U weight tile (1024 cols)

    with tile.TileContext(nc) as tc:
        with (
            tc.tile_pool(name="persist", bufs=1) as pp,
            tc.tile_pool(name="psum", bufs=8, space="PSUM") as psp,
        ):
            wgta = pp.tile([P, KD * S], f8, name="wgta", tag="wgta")
            wbta = pp.tile([P, KD * S], bf16, name="wbta", tag="wbta")
            wcta = pp.tile([P, MS * D], bf16, name="wcta", tag="wcta")
            biast = pp.tile([P, 4 + DT], f32, name="biast", tag="biast")
            xs8 = pp.tile([P, KD * L], f8, name="xs8", tag="xs8")
            xsa = pp.tile([P, KD * L], bf16, name="xsa", tag="xsa")
            ysta = pp.tile([P, DT * L], bf16, name="ysta", tag="ysta")

            # PE warm-up fodder (no DMA dependencies)
            wul = pp.tile([P, P], bf16, name="wul", tag="wul")

            # Input DMAs: one queue, exact consumption order, first
            # transfers halved so compute can chase the DMA.
            def dma_x8(q, parts=1):
                a, b = XOFF[q], XOFF[q + 1]
                step = (b - a) // parts
                for i in range(parts):
                    nc.sync.dma_start(
                        xs8[:, a + i * step:a + (i + 1) * step],
                        x8Q[:, a + i * step:a + (i + 1) * step])

            def dma_xb(q, parts=1):
                a, b = XOFF[q], XOFF[q + 1]
                step = (b - a) // parts
                for i in range(parts):
                    nc.sync.dma_start(
                        xsa[:, a + i * step:a + (i + 1) * step],
                        xQ[:, a + i * step:a + (i + 1) * step])

            # One HWDGE ring in exact consumption order: the 16 DMA
            # engines are a shared ~350-420 GB/s pipe (with a slow ~5us
            # ramp at kernel start), so parallel rings only dilute the
            # critical head transfers.  Only the tiny bias rides the
            # Scalar ring.
            nc.scalar.dma_start(biast[:], bias[:, :])
            # fine-grained head: the first matmul needs only wg8[m0,kp0]
            # (32KB) + x8[0][kp0] (128KB), so it can start ~9us while the
            # DMA pipe is still ramping; the PE then chases the stream.
            nc.sync.dma_start(wgta[:, :2 * P], wg[:, :2 * P])
            dma_x8(0, parts=4)
            nc.sync.dma_start(wgta[:, 2 * P:MH], wg[:, 2 * P:MH])
            nc.sync.dma_start(wgta[:, MH:], wg[:, MH:])
            nc.sync.dma_start(wbta[:, :MH], wb[:, :MH])
            dma_xb(0, parts=2)
            nc.sync.dma_start(wbta[:, MH:], wb[:, MH:])
            dma_x8(1)
            dma_xb(1, parts=2)
            nc.sync.dma_start(wcta[:], wc[:, :])
            for q in range(2, len(CHUNKS)):
                dma_x8(q)
                dma_xb(q)

            alpha = [pp.tile([P, L], f32, name=f"al{m}", tag=f"al{m}") for m in range(MS)]
            uu = [pp.tile([P, L], f32, name=f"uu{m}", tag=f"uu{m}") for m in range(MS)]
            hh = [pp.tile([P, L], bf16, name=f"hh{m}", tag=f"hh{m}") for m in range(MS)]

            if WARMUP_MMS:
                nc.gpsimd.memset(wul[:], 0.0)
                wps = psp.tile([P, 512], f32, name="wps", tag="ps")
                for i in range(WARMUP_MMS):
                    nc.tensor.matmul(
                        wps[:, :P], wul[:], wul[:],
                        start=(i == 0), stop=(i == WARMUP_MMS - 1),
                    )

            def emit_g(q):
                # gate GEMM: fp8 DoubleRow, K=256 per matmul
                o0, o1 = OFFS[q], OFFS[q + 1]
                cl = o1 - o0
                qs = slice(o0, o1)
                for m in range(MS):
                    ps = psp.tile([P, 512], f32, name="ps", tag="ps")
                    for kp in range(KP):
                        lhsT = wgta[:, m * MH + kp * 2 * P:m * MH + (kp + 1) * 2 * P
                                    ].rearrange("p (two m) -> p two m", two=2)
                        rhs = xs8[:, XOFF[q] + kp * 2 * cl:XOFF[q] + (kp + 1) * 2 * cl
                                  ].rearrange("p (two n) -> p two n", two=2)
                        nc.tensor.matmul(
                            ps[:, :cl], lhsT, rhs,
                            start=(kp == 0), stop=(kp == KP - 1),
                            perf_mode=DR,
                        )
                    nc.scalar.activation(
                        alpha[m][:, qs], ps[:, :cl], AF.Sigmoid,
                        bias=biast[:, m:m + 1], scale=1.0 / WG_SCALE,
                    )

            def emit_u(q):
                # input GEMM (bf16) + chunk-chained hardware scan
                o0, o1 = OFFS[q], OFFS[q + 1]
                cl = o1 - o0
                qs = slice(o0, o1)
                for m in range(MS):
                    ps = psp.tile([P, 512], f32, name="ps", tag="ps")
                    for k in range(KD):
                        nc.tensor.matmul(
                            ps[:, :cl],
                            wbta[:, m * MH + k * P:m * MH + (k + 1) * P],
                            xsa[:, XOFF[q] + k * cl:XOFF[q] + (k + 1) * cl],
                            start=(k == 0),
                            stop=(k == KD - 1),
                        )
                    nc.vector.tensor_scalar_add(
                        uu[m][:, qs], ps[:, :cl], biast[:, 2 + m:3 + m],
                    )
                # state = alpha*state + u
                for m in range(MS):
                    init = 0.0 if q == 0 else hh[m][:, o0 - 1:o0]
                    nc.vector.tensor_tensor_scan(
                        hh[m][:, qs], alpha[m][:, qs], uu[m][:, qs],
                        init, OP.mult, OP.add,
                    )

            def emit_y(q):
                o0, o1 = OFFS[q], OFFS[q + 1]
                cl = o1 - o0
                qs = slice(o0, o1)
                last = q == len(CHUNKS) - 1
                for t in range(DT):
                    ps = psp.tile([P, 512], f32, name="psy", tag="ps")
                    for m in range(MS):
                        nc.tensor.matmul(
                            ps[:, :cl],
                            wcta[:, m * D + t * P:m * D + (t + 1) * P],
                            hh[m][:, qs],
                            start=(m == 0),
                            stop=(m == MS - 1),
                        )
                    dst = ysta[:, YOFF[q] + t * cl:YOFF[q] + (t + 1) * cl]
                    bc = biast[:, 4 + t:5 + t]
                    if last and t == DT - 1:
                        # the very last eviction gates the final writeback:
                        # split it across both engines so it lands sooner
                        hl = cl // 2
                        nc.scalar.activation(
                            dst[:, :hl], ps[:, :hl], AF.Identity, bias=bc, scale=1.0
                        )
                        nc.vector.tensor_scalar_add(dst[:, hl:], ps[:, hl:cl], bc)
                    elif t % 2 == 0:
                        nc.scalar.activation(dst, ps[:, :cl], AF.Identity, bias=bc, scale=1.0)
                    else:
                        nc.vector.tensor_scalar_add(dst, ps[:, :cl], bc)
                    if t == DT // 2 - 1:
                        # first-half writeback starts while the second half
                        # of this chunk's Y GEMMs still run
                        nc.sync.dma_start(
                            y[:, YOFF[q]:YOFF[q] + DT // 2 * cl],
                            ysta[:, YOFF[q]:YOFF[q] + DT // 2 * cl],
                        )
                # final chunk: second half goes out on the otherwise-idle
                # Scalar HWDGE ring so the two halves drain in parallel
                eng = nc.scalar if q == len(CHUNKS) - 1 else nc.sync
                eng.dma_start(
                    y[:, YOFF[q] + DT // 2 * cl:YOFF[q + 1]],
                    ysta[:, YOFF[q] + DT // 2 * cl:YOFF[q + 1]],
                )

            # software pipeline: Y GEMMs run one chunk behind the G/U
            # GEMMs (never waiting on the scan), interleaved between U and
            # G so Y work can fill x-DMA wait gaps.  The final chunk's Y
            # is pulled as close to its scan as possible.
            # PE order: g0 u0 g1 y0 u1 g2 y1 u2 g3 y2 u3 y3 g4 u4 y4
            emit_g(0)
            emit_u(0)
            for q in range(1, len(CHUNKS)):
                emit_g(q)
                emit_u(q)
                emit_y(q - 1)
            emit_y(len(CHUNKS) - 1)

    nc.finalize()
    return nc


def _get_nc():
    if "nc" not in _NC_CACHE:
        _NC_CACHE["nc"] = _build_nc()
    return _NC_CACHE["nc"]


def _make_in_maps(x, Wg, bg, WB, bB, WC, bC):
    import ml_dtypes

    bf16 = ml_dtypes.bfloat16
    f8 = ml_dtypes.float8_e4m3
    x = np.asarray(x, dtype=np.float32)
    # fp8 gate weights, DoubleRow layout: (p, m, kp, two, j) ->
    # Wg.T[(2kp+two)*P+p, m*P+j] * WG_SCALE
    wg8P = np.ascontiguousarray(
        np.clip(np.asarray(Wg, dtype=np.float32).T * WG_SCALE, -240, 240)
        .astype(f8)
        .reshape(KP, 2, P, MS, P).transpose(2, 3, 0, 1, 4).reshape(P, KD * S)
    )
    # bf16 U weights m-major: (p, m, k, j) -> WB.T[k*P+p, m*P+j]
    wbP = np.ascontiguousarray(
        np.asarray(WB, dtype=np.float32).T.astype(bf16)
        .reshape(KD, P, MS, P).transpose(1, 2, 0, 3).reshape(P, MS * KD * P)
    )
    wcP = np.ascontiguousarray(
        np.asarray(WC, dtype=np.float32).T.astype(bf16)
        .reshape(MS, P, D).transpose(1, 0, 2).reshape(P, MS * D)
    )
    bias = np.zeros((P, 4 + DT), dtype=np.float32)
    bias[:, 0] = np.asarray(bg, dtype=np.float32)[0:P]
    bias[:, 1] = np.asarray(bg, dtype=np.float32)[P:2 * P]
    bias[:, 2] = np.asarray(bB, dtype=np.float32)[0:P]
    bias[:, 3] = np.asarray(bB, dtype=np.float32)[P:2 * P]
    bias[:, 4:] = np.asarray(bC, dtype=np.float32).reshape(DT, P).T
    in_maps = []
    for b in range(NCORES):
        xt = np.ascontiguousarray(x[b].T)          # [D, L] f32
        xkb = xt.astype(bf16).reshape(KD, P, L)    # [k, p, l]
        xk8 = np.clip(xt, -240, 240).astype(f8).reshape(KD, P, L)
        bb, b8 = [], []
        for q in range(len(CHUNKS)):
            sl = slice(OFFS[q], OFFS[q + 1])
            # bf16 block: (p, k, l)
            bb.append(xkb[:, :, sl].transpose(1, 0, 2).reshape(P, -1))
            # fp8 block: (p, kp, two, l)
            b8.append(
                xk8[:, :, sl].reshape(KP, 2, P, -1).transpose(2, 0, 1, 3).reshape(P, -1)
            )
        in_maps.append({
            "xQ": np.ascontiguousarray(np.concatenate(bb, axis=1)),
            "x8Q": np.ascontiguousarray(np.concatenate(b8, axis=1)),
            "wg8P": wg8P,
            "wbP": wbP,
            "wcP": wcP,
            "biasP": bias,
        })
    return in_maps


def _run(in_maps, **kwargs):
    from concourse.bass_utils import run_bass_kernel_spmd

    nc = _get_nc()
    return run_bass_kernel_spmd(nc, in_maps, list(range(NCORES)), **kwargs)


def kernel(x, Wg, bg, WB, bB, WC, bC):
    res = _run(_make_in_maps(x, Wg, bg, WB, bB, WC, bC))
    out = np.empty((NCORES, L, D), dtype=np.float32)
    for b in range(NCORES):
        yq = np.asarray(res.results[b]["yQ"])
        for q in range(len(CHUNKS)):
            o0, o1 = OFFS[q], OFFS[q + 1]
            cl = o1 - o0
            blk = yq[:, YOFF[q]:YOFF[q + 1]].reshape(P, DT, cl)
            # yQ[p, t, l] = y[o0+l, t*P+p]
            out[b, o0:o1, :] = blk.transpose(2, 1, 0).reshape(cl, D).astype(np.float32)
    return out
